# revision 25
# baseline (speedup 1.0000x reference)
"""AttentionPooling segment-reduce kernel for 8 Trainium2 NeuronCores.

Math (reference):
    k = x @ key_w.T + key_b            # [N, 256] -> heads [N, 4, 64]
    v = x @ value_w.T + value_b
    attn   = einsum('hd,nhd->nh', query, k) * SCALE
    w      = exp(attn)
    wsum   = segment_sum(w)[batch]
    out[b] = segment_sum(w/(wsum+EPS) * v)

Algebraic restructuring (exact):
    attn[n,h] = qt[:,h] . x[n] + sc[h],  qt = SCALE*(key_w^T q per head),
                                         sc = SCALE*(q . key_b per head)
    w = exp(attn) = g[h]*wt[n,h],  wt = exp(qt . x),  g = exp(sc)
    v' = x @ value_w.T                 (bias deferred to segment level)
    St[b,f] = sum_{n in b} wt[n,h(f)] v'[n,f];  dt[b,h] = sum_{n in b} wt[n,h]
    out[b,f] = (St[b,f] + dt[b,h]*value_b[f]) / (dt[b,h] + EPS/g[h])

Device mapping: core c owns segments [c*512,(c+1)*512) split into 4 windows of
128 segments; window nodes padded to 128-multiples. Per 128-node tile:
 - PE: fused projection psum[nodes,260] = xT_tile.T @ [Wv^T | qt] (fp16 in,
   fp32 accum), then segment reduce psum_s[segs,260] += onehot.T @ u.
 - ACT: exp of the 4 attn columns (batched over the tile group).
 - DVE: u[:,0:256] = psum[:,0:256] * wt (head-broadcast), one batched op/group.
One-hot node->segment matrices are precomputed on the host (exact 0/1 fp16)
and streamed alongside x^T, so no on-device index compute is needed.
Window epilogue (DVE): out = (St + dt*bv) / (dt + eps/g), DMA to the core's
output rows. Host pre-transposes x to [256, N] fp16 so the contraction dim
lands on SBUF partitions.
"""

from contextlib import ExitStack

import numpy as np

N = 262144
DIM = 256
H = 4
HD = 64
B = 4096
SCALE = HD ** (-0.5)
EPS = 1e-8

NCORES = 8
SEGS_PER_CORE = B // NCORES          # 512
WPC = 4                              # windows per core
WSEG = SEGS_PER_CORE // WPC          # 128 segments per window
GRP = 2                              # node-tiles per PSUM group
CHUNK = 1024                         # x columns per DMA chunk

TRACE = False                        # test harness can flip for profiling
LAST_RESULT = None

_cache = {}


def _build(tw: int):
    """Build + compile the SPMD program for tw node-tiles per window."""
    import concourse.tile as tile
    from concourse import bacc, mybir

    F32 = mybir.dt.float32
    F16 = mybir.dt.float16
    Alu = mybir.AluOpType
    Act = mybir.ActivationFunctionType

    P = WPC * tw * 128

    nc = bacc.Bacc("TRN2", target_bir_lowering=False, debug=False,
                   num_devices=NCORES)

    pk_d = nc.dram_tensor("pk", [128, 3 * P], F16, kind="ExternalInput").ap()
    wq_d = nc.dram_tensor("wq", [128, 520], F16, kind="ExternalInput").ap()
    cst_d = nc.dram_tensor("cst", [128, 260], F32, kind="ExternalInput").ap()
    out_d = nc.dram_tensor("out", [SEGS_PER_CORE, 256], F32,
                           kind="ExternalOutput").ap()

    with tile.TileContext(nc) as tc, ExitStack() as ctx:
        consts = ctx.enter_context(tc.tile_pool(name="consts", bufs=1))
        xin = ctx.enter_context(tc.tile_pool(name="xin", bufs=6))
        up = ctx.enter_context(tc.tile_pool(name="up", bufs=4))
        fxp = ctx.enter_context(tc.tile_pool(name="fxp", bufs=2))
        pp = ctx.enter_context(tc.tile_pool(name="pp", bufs=3, space="PSUM"))
        sp = ctx.enter_context(tc.tile_pool(name="sp", bufs=2, space="PSUM"))

        # PE warm-up: ~4.5us of dummy matmuls on zeros, issued with no DMA
        # dependency so they run during the initial input-chunk DMA wait and
        # flip the HAM clock gate to 2.4 GHz before real work arrives.
        wtile = consts.tile([128, 128], F16, tag="wtile")
        nc.vector.memset(wtile[:], 0.0)
        wpsum = pp.tile([128, 2 * 512], F32, tag="pp")
        for _ in range(40):
            nc.tensor.matmul(wpsum[:, 0:128], wtile[:], wtile[:],
                             start=True, stop=True)

        wqpk = consts.tile([128, 520], F16, tag="wqpk")
        cst = consts.tile([128, 260], F32, tag="cst")
        nc.sync.dma_start(wqpk[:], wq_d)
        nc.sync.dma_start(cst[:], cst_d)
        wq0 = wqpk[:, 0:260]
        wq1 = wqpk[:, 260:520]
        bvrep = cst[:, 0:256]
        epsg = cst[:, 256:260]

        pkt = None
        for w in range(WPC):
            psum_s = sp.tile([128, 260], F32, tag="ps")
            for g0 in range(0, tw, GRP):
                gsz = min(GRP, tw - g0)
                psum4 = pp.tile([128, gsz * 512], F32, tag="pp")
                u4 = up.tile([128, gsz * 260], F16, tag="u4")
                ohview = []
                for b in range(gsz):
                    t = w * tw + g0 + b          # core-local tile index
                    col = t * 128
                    if col % CHUNK == 0:
                        cw = min(CHUNK, P - col)
                        pkt = xin.tile([128, 3 * CHUNK], F16, tag="pkt")
                        if col == 0:
                            d3 = pkt[:].rearrange("p (pl c) -> p pl c", pl=3)
                            s3 = (pk_d[:, 0:3 * cw]
                                  .rearrange("p (pl c) -> p pl c", pl=3))
                            nc.sync.dma_start(d3[:, :, 0:256], s3[:, :, 0:256])
                            nc.sync.dma_start(d3[:, :, 256:cw],
                                              s3[:, :, 256:cw])
                        else:
                            nc.sync.dma_start(
                                pkt[:, 0:3 * cw],
                                pk_d[:, 3 * col:3 * col + 3 * cw])
                    o = col % CHUNK
                    ps = psum4[:, b * 512:b * 512 + 260]
                    nc.tensor.matmul(ps, pkt[:, o:o + 128], wq0,
                                     start=True, stop=False)
                    nc.tensor.matmul(ps, pkt[:, CHUNK + o:CHUNK + o + 128],
                                     wq1, start=False, stop=True)
                    ohview.append(pkt[:, 2 * CHUNK + o:2 * CHUNK + o + 128])

                p3 = psum4[:].rearrange("p (b c) -> p b c", c=512)
                u3 = u4[:].rearrange("p (b c) -> p b c", c=260)
                nc.scalar.activation(u3[:, :, 256:260], p3[:, :, 256:260],
                                     Act.Exp)
                in0 = p3[:, :, 0:256].rearrange("p b (h d) -> p b h d", h=H)
                in1 = (u3[:, :, 256:260].unsqueeze(3)
                       .broadcast_to([128, gsz, H, HD]))
                o4 = u3[:, :, 0:256].rearrange("p b (h d) -> p b h d", h=H)
                nc.vector.tensor_tensor(o4, in0, in1, Alu.mult)

                for b in range(gsz):
                    t = w * tw + g0 + b
                    nc.tensor.matmul(psum_s[:], ohview[b],
                                     u4[:, b * 260:(b + 1) * 260],
                                     start=(t == w * tw),
                                     stop=(t == w * tw + tw - 1))

            # ---- window epilogue ----
            dsum = fxp.tile([128, 4], F32, tag="dsum")
            nc.vector.tensor_tensor(dsum[:], psum_s[:, 256:260], epsg,
                                    Alu.add)
            rec = fxp.tile([128, 4], F32, tag="rec")
            nc.vector.reciprocal(rec[:], dsum[:])
            t1 = fxp.tile([128, 256], F32, tag="t1")
            bv3 = bvrep.rearrange("p (h d) -> p h d", h=H)
            dt3 = (psum_s[:, 256:260].unsqueeze(2)
                   .broadcast_to([128, H, HD]))
            nc.vector.tensor_tensor(
                t1[:].rearrange("p (h d) -> p h d", h=H), bv3, dt3, Alu.mult)
            t2 = fxp.tile([128, 256], F32, tag="t2")
            nc.vector.tensor_tensor(t2[:], psum_s[:, 0:256], t1[:], Alu.add)
            outt = fxp.tile([128, 256], F32, tag="outt")
            rec3 = rec[:].unsqueeze(2).broadcast_to([128, H, HD])
            nc.vector.tensor_tensor(
                outt[:].rearrange("p (h d) -> p h d", h=H),
                t2[:].rearrange("p (h d) -> p h d", h=H), rec3, Alu.mult)
            nc.sync.dma_start(out_d[w * 128:(w + 1) * 128, :], outt[:])

    nc.compile()
    return nc


def kernel(x, batch, query, key_w, key_b, value_w, value_b):
    global LAST_RESULT
    from concourse.bass_utils import run_bass_kernel_spmd

    x = np.asarray(x, dtype=np.float32)
    batch = np.asarray(batch).astype(np.int64)
    query = np.asarray(query, dtype=np.float32)
    key_w = np.asarray(key_w, dtype=np.float32)
    key_b = np.asarray(key_b, dtype=np.float32)
    value_w = np.asarray(value_w, dtype=np.float32)
    value_b = np.asarray(value_b, dtype=np.float32)

    # ---- host-side planning ----
    counts = np.bincount(batch, minlength=B)
    cum = np.zeros(B + 1, np.int64)
    cum[1:] = np.cumsum(counts)
    nwin = NCORES * WPC
    wstart = cum[np.arange(nwin) * WSEG]
    wend = cum[(np.arange(nwin) + 1) * WSEG]
    tiles_w = (wend - wstart + 127) // 128
    tw = int(tiles_w.max())
    tw += tw % 2                      # keep P a multiple of CHUNK
    P = WPC * tw * 128

    # ---- shared constants ----
    wqf = np.zeros((256, 260), np.float32)
    wqf[:, 0:256] = value_w.T
    qt = (key_w.reshape(H, HD, DIM) * query[:, :, None]).sum(axis=1)  # [H,256]
    wqf[:, 256:260] = SCALE * qt.T
    wq = np.concatenate([wqf[0:128], wqf[128:256]],
                        axis=1).astype(np.float16)          # [128, 520]
    sc = SCALE * (query * key_b.reshape(H, HD)).sum(axis=1)           # [H]
    g = np.exp(sc).astype(np.float32)
    cst = np.zeros((128, 260), np.float32)
    cst[:, 0:256] = value_b
    cst[:, 256:260] = EPS / g

    # ---- per-core shards ----
    in_maps = []
    for c in range(NCORES):
        pk = np.zeros((128, 3 * P), np.float16)
        pk3 = pk.reshape(128, P // CHUNK, 3, CHUNK)   # [p, chunk, plane, col]
        xTp = np.zeros((256, P), np.float16)
        ohp = np.zeros((128, P), np.float16)
        oh_t = ohp.reshape(128, P // 128, 128)        # [p, tile, j]
        for w in range(WPC):
            m = c * WPC + w
            ns, ne = int(wstart[m]), int(wend[m])
            L = ne - ns
            col0 = w * tw * 128
            xTp[:, col0:col0 + L] = x[ns:ne, :].T.astype(np.float16)
            j = (batch[ns:ne] - m * WSEG).astype(np.int64)
            node = np.arange(L) + col0
            oh_t[node % 128, node // 128, j] = np.float16(1.0)
        xc = xTp.reshape(256, P // CHUNK, CHUNK)
        pk3[:, :, 0, :] = xc[0:128]
        pk3[:, :, 1, :] = xc[128:256]
        pk3[:, :, 2, :] = ohp.reshape(128, P // CHUNK, CHUNK)
        in_maps.append({"pk": pk, "wq": wq, "cst": cst})

    if tw not in _cache:
        _cache[tw] = _build(tw)
    nc = _cache[tw]

    res = run_bass_kernel_spmd(nc, in_maps, core_ids=list(range(NCORES)),
                               trace=TRACE)
    LAST_RESULT = res
    return np.concatenate([r["out"] for r in res.results], axis=0)


# revision 26
# speedup vs baseline: 1.0074x; 1.0074x over previous
"""AttentionPooling segment-reduce kernel for 8 Trainium2 NeuronCores.

Math (reference):
    k = x @ key_w.T + key_b            # [N, 256] -> heads [N, 4, 64]
    v = x @ value_w.T + value_b
    attn   = einsum('hd,nhd->nh', query, k) * SCALE
    w      = exp(attn)
    wsum   = segment_sum(w)[batch]
    out[b] = segment_sum(w/(wsum+EPS) * v)

Algebraic restructuring (exact):
    attn[n,h] = qt[:,h] . x[n] + sc[h],  qt = SCALE*(key_w^T q per head),
                                         sc = SCALE*(q . key_b per head)
    w = exp(attn) = g[h]*wt[n,h],  wt = exp(qt . x),  g = exp(sc)
    v' = x @ value_w.T                 (bias deferred to segment level)
    St[b,f] = sum_{n in b} wt[n,h(f)] v'[n,f];  dt[b,h] = sum_{n in b} wt[n,h]
    out[b,f] = (St[b,f] + dt[b,h]*value_b[f]) / (dt[b,h] + EPS/g[h])

Device mapping: core c owns segments [c*512,(c+1)*512) split into 4 windows of
128 segments; window nodes padded to 128-multiples. Per 128-node tile:
 - PE: fused projection psum[nodes,260] = xT_tile.T @ [Wv^T | qt] (fp16 in,
   fp32 accum), then segment reduce psum_s[segs,260] += onehot.T @ u.
 - ACT: exp of the 4 attn columns (batched over the tile group).
 - DVE: u[:,0:256] = psum[:,0:256] * wt (head-broadcast), one batched op/group.
One-hot node->segment matrices are precomputed on the host (exact 0/1 fp16)
and streamed alongside x^T, so no on-device index compute is needed.
Window epilogue (DVE): out = (St + dt*bv) / (dt + eps/g), DMA to the core's
output rows. Host pre-transposes x to [256, N] fp16 so the contraction dim
lands on SBUF partitions.
"""

from contextlib import ExitStack

import numpy as np

N = 262144
DIM = 256
H = 4
HD = 64
B = 4096
SCALE = HD ** (-0.5)
EPS = 1e-8

NCORES = 8
SEGS_PER_CORE = B // NCORES          # 512
WPC = 4                              # windows per core
WSEG = SEGS_PER_CORE // WPC          # 128 segments per window
GRP = 2                              # node-tiles per PSUM group
CHUNK = 1024                         # x columns per DMA chunk

TRACE = False                        # test harness can flip for profiling
LAST_RESULT = None

_cache = {}


def _build(tw: int):
    """Build + compile the SPMD program for tw node-tiles per window."""
    import concourse.tile as tile
    from concourse import bacc, mybir

    F32 = mybir.dt.float32
    F16 = mybir.dt.float16
    Alu = mybir.AluOpType
    Act = mybir.ActivationFunctionType

    P = WPC * tw * 128

    nc = bacc.Bacc("TRN2", target_bir_lowering=False, debug=False,
                   num_devices=NCORES)

    pk_d = nc.dram_tensor("pk", [128, 3 * P], F16, kind="ExternalInput").ap()
    wq_d = nc.dram_tensor("wq", [128, 520], F16, kind="ExternalInput").ap()
    cst_d = nc.dram_tensor("cst", [128, 260], F32, kind="ExternalInput").ap()
    out_d = nc.dram_tensor("out", [SEGS_PER_CORE, 256], F32,
                           kind="ExternalOutput").ap()

    with tile.TileContext(nc, pool_alloc_mode="queue") as tc, \
            ExitStack() as ctx:
        consts = ctx.enter_context(tc.tile_pool(name="consts", bufs=1))
        xin = ctx.enter_context(tc.tile_pool(name="xin", bufs=6))
        up = ctx.enter_context(tc.tile_pool(name="up", bufs=4))
        fxp = ctx.enter_context(tc.tile_pool(name="fxp", bufs=2))
        pp = ctx.enter_context(tc.tile_pool(name="pp", bufs=3, space="PSUM"))
        sp = ctx.enter_context(tc.tile_pool(name="sp", bufs=2, space="PSUM"))

        # PE warm-up: ~4.5us of dummy matmuls on zeros, issued with no DMA
        # dependency so they run during the initial input-chunk DMA wait and
        # flip the HAM clock gate to 2.4 GHz before real work arrives.
        wtile = consts.tile([128, 128], F16, tag="wtile")
        nc.vector.memset(wtile[:], 0.0)
        wpsum = pp.tile([128, 2 * 512], F32, tag="pp")
        for _ in range(40):
            nc.tensor.matmul(wpsum[:, 0:128], wtile[:], wtile[:],
                             start=True, stop=True)

        wqpk = consts.tile([128, 520], F16, tag="wqpk")
        cst = consts.tile([128, 260], F32, tag="cst")
        nc.sync.dma_start(wqpk[:], wq_d)
        nc.sync.dma_start(cst[:], cst_d)
        wq0 = wqpk[:, 0:260]
        wq1 = wqpk[:, 260:520]
        bvrep = cst[:, 0:256]
        epsg = cst[:, 256:260]

        pkt = None
        for w in range(WPC):
            psum_s = sp.tile([128, 260], F32, tag="ps")
            for g0 in range(0, tw, GRP):
                gsz = min(GRP, tw - g0)
                psum4 = pp.tile([128, gsz * 512], F32, tag="pp")
                u4 = up.tile([128, gsz * 260], F16, tag="u4")
                ohview = []
                for b in range(gsz):
                    t = w * tw + g0 + b          # core-local tile index
                    col = t * 128
                    if col % CHUNK == 0:
                        cw = min(CHUNK, P - col)
                        pkt = xin.tile([128, 3 * CHUNK], F16, tag="pkt")
                        if col == 0:
                            d3 = pkt[:].rearrange("p (pl c) -> p pl c", pl=3)
                            s3 = (pk_d[:, 0:3 * cw]
                                  .rearrange("p (pl c) -> p pl c", pl=3))
                            nc.sync.dma_start(d3[:, :, 0:256], s3[:, :, 0:256])
                            nc.sync.dma_start(d3[:, :, 256:cw],
                                              s3[:, :, 256:cw])
                        else:
                            nc.sync.dma_start(
                                pkt[:, 0:3 * cw],
                                pk_d[:, 3 * col:3 * col + 3 * cw])
                    o = col % CHUNK
                    ps = psum4[:, b * 512:b * 512 + 260]
                    nc.tensor.matmul(ps, pkt[:, o:o + 128], wq0,
                                     start=True, stop=False)
                    nc.tensor.matmul(ps, pkt[:, CHUNK + o:CHUNK + o + 128],
                                     wq1, start=False, stop=True)
                    ohview.append(pkt[:, 2 * CHUNK + o:2 * CHUNK + o + 128])

                p3 = psum4[:].rearrange("p (b c) -> p b c", c=512)
                u3 = u4[:].rearrange("p (b c) -> p b c", c=260)
                nc.scalar.activation(u3[:, :, 256:260], p3[:, :, 256:260],
                                     Act.Exp)
                in0 = p3[:, :, 0:256].rearrange("p b (h d) -> p b h d", h=H)
                in1 = (u3[:, :, 256:260].unsqueeze(3)
                       .broadcast_to([128, gsz, H, HD]))
                o4 = u3[:, :, 0:256].rearrange("p b (h d) -> p b h d", h=H)
                nc.vector.tensor_tensor(o4, in0, in1, Alu.mult)

                for b in range(gsz):
                    t = w * tw + g0 + b
                    nc.tensor.matmul(psum_s[:], ohview[b],
                                     u4[:, b * 260:(b + 1) * 260],
                                     start=(t == w * tw),
                                     stop=(t == w * tw + tw - 1))

            # ---- window epilogue ----
            dsum = fxp.tile([128, 4], F32, tag="dsum")
            nc.vector.tensor_tensor(dsum[:], psum_s[:, 256:260], epsg,
                                    Alu.add)
            rec = fxp.tile([128, 4], F32, tag="rec")
            nc.vector.reciprocal(rec[:], dsum[:])
            t1 = fxp.tile([128, 256], F32, tag="t1")
            bv3 = bvrep.rearrange("p (h d) -> p h d", h=H)
            dt3 = (psum_s[:, 256:260].unsqueeze(2)
                   .broadcast_to([128, H, HD]))
            nc.vector.tensor_tensor(
                t1[:].rearrange("p (h d) -> p h d", h=H), bv3, dt3, Alu.mult)
            t2 = fxp.tile([128, 256], F32, tag="t2")
            nc.vector.tensor_tensor(t2[:], psum_s[:, 0:256], t1[:], Alu.add)
            outt = fxp.tile([128, 256], F32, tag="outt")
            rec3 = rec[:].unsqueeze(2).broadcast_to([128, H, HD])
            nc.vector.tensor_tensor(
                outt[:].rearrange("p (h d) -> p h d", h=H),
                t2[:].rearrange("p (h d) -> p h d", h=H), rec3, Alu.mult)
            nc.sync.dma_start(out_d[w * 128:(w + 1) * 128, :], outt[:])

    nc.compile()
    return nc


def kernel(x, batch, query, key_w, key_b, value_w, value_b):
    global LAST_RESULT
    from concourse.bass_utils import run_bass_kernel_spmd

    x = np.asarray(x, dtype=np.float32)
    batch = np.asarray(batch).astype(np.int64)
    query = np.asarray(query, dtype=np.float32)
    key_w = np.asarray(key_w, dtype=np.float32)
    key_b = np.asarray(key_b, dtype=np.float32)
    value_w = np.asarray(value_w, dtype=np.float32)
    value_b = np.asarray(value_b, dtype=np.float32)

    # ---- host-side planning ----
    counts = np.bincount(batch, minlength=B)
    cum = np.zeros(B + 1, np.int64)
    cum[1:] = np.cumsum(counts)
    nwin = NCORES * WPC
    wstart = cum[np.arange(nwin) * WSEG]
    wend = cum[(np.arange(nwin) + 1) * WSEG]
    tiles_w = (wend - wstart + 127) // 128
    tw = int(tiles_w.max())
    tw += tw % 2                      # keep P a multiple of CHUNK
    P = WPC * tw * 128

    # ---- shared constants ----
    wqf = np.zeros((256, 260), np.float32)
    wqf[:, 0:256] = value_w.T
    qt = (key_w.reshape(H, HD, DIM) * query[:, :, None]).sum(axis=1)  # [H,256]
    wqf[:, 256:260] = SCALE * qt.T
    wq = np.concatenate([wqf[0:128], wqf[128:256]],
                        axis=1).astype(np.float16)          # [128, 520]
    sc = SCALE * (query * key_b.reshape(H, HD)).sum(axis=1)           # [H]
    g = np.exp(sc).astype(np.float32)
    cst = np.zeros((128, 260), np.float32)
    cst[:, 0:256] = value_b
    cst[:, 256:260] = EPS / g

    # ---- per-core shards ----
    in_maps = []
    for c in range(NCORES):
        pk = np.zeros((128, 3 * P), np.float16)
        pk3 = pk.reshape(128, P // CHUNK, 3, CHUNK)   # [p, chunk, plane, col]
        xTp = np.zeros((256, P), np.float16)
        ohp = np.zeros((128, P), np.float16)
        oh_t = ohp.reshape(128, P // 128, 128)        # [p, tile, j]
        for w in range(WPC):
            m = c * WPC + w
            ns, ne = int(wstart[m]), int(wend[m])
            L = ne - ns
            col0 = w * tw * 128
            xTp[:, col0:col0 + L] = x[ns:ne, :].T.astype(np.float16)
            j = (batch[ns:ne] - m * WSEG).astype(np.int64)
            node = np.arange(L) + col0
            oh_t[node % 128, node // 128, j] = np.float16(1.0)
        xc = xTp.reshape(256, P // CHUNK, CHUNK)
        pk3[:, :, 0, :] = xc[0:128]
        pk3[:, :, 1, :] = xc[128:256]
        pk3[:, :, 2, :] = ohp.reshape(128, P // CHUNK, CHUNK)
        in_maps.append({"pk": pk, "wq": wq, "cst": cst})

    if tw not in _cache:
        _cache[tw] = _build(tw)
    nc = _cache[tw]

    res = run_bass_kernel_spmd(nc, in_maps, core_ids=list(range(NCORES)),
                               trace=TRACE)
    LAST_RESULT = res
    return np.concatenate([r["out"] for r in res.results], axis=0)
